# revision 77
# baseline (speedup 1.0000x reference)
"""GAT model Bass/Tile kernel for TRN2 (self-contained, fine-software-pipelined).

Per core: 512 graphs as 256 pairs (128 nodes / 112 edges per pair),
processed in octets (8 pairs). Each octet's program is split into ~45
fine-grained stages (every cross-engine hop crosses a stage boundary)
and ~4 octets are in flight, staggered START_IVL stages apart, so all
engines stay fed. Layer 1 runs gather-first (gather 16-dim x at edge
endpoints, then project), which skips the layer-1 PSUM h copies.
Graph-structure tensors (one-hot gather/scatter matrices, edge-attr
projections, self-loop means) are host-precomputed and streamed by DMA.
GPSIMD cannot touch PSUM, so PSUM-reading elementwise work is placed on
DVE/ACT and GPSIMD handles SBUF-only multiplies (self messages, logits).
"""
import numpy as np
from contextlib import ExitStack

import concourse.bass as bass
import concourse.tile as tile
from concourse import bacc, mybir
from concourse.bass_utils import run_bass_kernel_spmd

F32 = mybir.dt.float32
I32 = mybir.dt.int32
FP8 = mybir.dt.float8e4

B, A, OBS = 4096, 8, 56
P = 64
H, HID, HC = 8, 32, 256
IN, OUT = 16, 2
NCORES = 8
GPC = B // NCORES
EPP = 2 * OBS              # 112 edges per pair
ALU = mybir.AluOpType
ACTF = mybir.ActivationFunctionType

SC_, ASD_, DEN_ = 0, 208, 272   # psc column regions (f32)
START_IVL = 10


def build(npairs: int, vdt=mybir.dt.bfloat16, num_devices: int = NCORES,
          start_ivl: int = START_IVL):
    assert npairs % 8 == 0
    noct = npairs // 8
    nc = bacc.Bacc("TRN2", target_bir_lowering=False, debug=False,
                   num_devices=num_devices)

    def din(name, shape, dt):
        return nc.dram_tensor(name, shape, dt, kind="ExternalInput").ap()

    # per-octet streams are packed octet-major so each DMA is contiguous;
    # one-hot structure matrices are exact in fp8 (halves their DMA bytes)
    sdfr = din("sdfr", [noct * 128, 1792], FP8)  # [sblk|dblk] x 8 pairs
    xnr = din("xnr", [noct * 128, 128], vdt)     # x per node
    dttae = din("dttae", [noct * 112, 1024], FP8)
    ddr = din("ddr", [noct * 112, 896], FP8)     # dst-coincidence (L1 denom)
    eawr = din("eawr", [noct * 112, 192], vdt)   # ea*we outer products
    elwr = din("elwr", [noct * 128, 128], vdt)   # ea_loop*we outer products
    w1vr = din("w1vr", [IN, 256], vdt)     # W1
    w1sr = din("w1sr", [IN, 16], vdt)      # [Ps1|Pd1]
    w2v = din("w2v", [128, 512], vdt)
    w2s = din("w2s", [128, 48], vdt)
    w3v = din("w3v", [128, 512], vdt)
    w3s = din("w3s", [128, 48], vdt)
    fc1a = din("fc1a", [128, HC], vdt)
    fc1g = din("fc1g", [128, HC], vdt)
    fc1b = din("fc1b", [128, 1], F32)
    fc2w = din("fc2w", [128, OUT], vdt)
    fc2b = din("fc2b", [OUT, 1], F32)
    ident = din("ident", [128, 128], vdt)

    out_d = nc.dram_tensor("out", [OUT, npairs * 16], F32,
                           kind="ExternalOutput").ap()

    with tile.TileContext(nc) as tc, ExitStack() as ctx:
        ctx.enter_context(nc.allow_low_precision(reason="bf16 attention probs"))
        cpool = ctx.enter_context(tc.tile_pool(name="const", bufs=1))
        wk = ctx.enter_context(tc.tile_pool(name="work", bufs=4))
        eb = ctx.enter_context(tc.tile_pool(name="edges", bufs=6))
        ps = ctx.enter_context(tc.tile_pool(name="psum", bufs=1, space="PSUM"))

        def cload(ap, tag):
            t = cpool.tile(list(ap.shape), ap.dtype, tag=tag)
            nc.sync.dma_start(t[:], ap[:, :])
            return t

        deferred = []

        def cload_deferred(ap, tag):
            # alloc now, DMA after octet 0's stream DMAs so the big
            # constants (not needed until mid-octet) don't delay the
            # pipeline fill on the DMA queues
            t = cpool.tile(list(ap.shape), ap.dtype, tag=tag)
            deferred.append((t, ap))
            return t

        c_w1vr, c_w1sr = cload(w1vr, "w1vr"), cload(w1sr, "w1sr")
        c_w2v, c_w2s = cload_deferred(w2v, "w2v"), cload_deferred(w2s, "w2s")
        c_w3v, c_w3s = cload_deferred(w3v, "w3v"), cload_deferred(w3s, "w3s")
        c_fc1a, c_fc1g = (cload_deferred(fc1a, "fc1a"),
                          cload_deferred(fc1g, "fc1g"))
        c_fc1b, c_fc2w, c_fc2b = (cload(fc1b, "fc1b"), cload(fc2w, "fc2w"),
                                  cload(fc2b, "fc2b"))
        c_id = cload_deferred(ident, "ident")

        out_acc = cpool.tile([OUT, npairs * 16], F32, tag="out_acc")

        wv = {2: [c_w2v[:, 0:256], c_w2v[:, 256:512]],
              3: [c_w3v[:, 0:256], c_w3v[:, 256:512]]}
        wsc = {2: [c_w2s[:, 0:24], c_w2s[:, 24:48]],
               3: [c_w3s[:, 0:24], c_w3s[:, 24:48]]}

        def softmax_stages(li, psc, dblk, dtt, sA_s1, eaw_t, elw_t, ddm):
            """Stages f3..f8 shared by all layers: logits -> pn/psn.
            The ea*we edge-attr term is host-precomputed (eaw/elw) and
            added on DVE here instead of rank-1 PE matmuls. For L1 (no
            self loops) per-edge denominators come from one DD matmul
            (dst-coincidence), skipping the scatter+gather pair."""
            self_loops = li > 1
            # f3: add ea*we term + leaky-relu logits; exp'd in one op
            lgs = wk.tile([128, 128], vdt, tag="lgs")
            lg = wk.tile([112, 64], vdt, tag="lg")
            nc.vector.tensor_tensor(
                lg[:], psc[0:112, ASD_:ASD_ + 64],
                eaw_t[0:112, (li - 1) * 64:li * 64], ALU.add)
            nc.vector.scalar_tensor_tensor(
                lgs[0:112, 0:64], lg[:], 0.2, lg[:], ALU.mult, ALU.max)
            if self_loops:
                slg = wk.tile([128, 64], vdt, tag="slg")
                slgv = slg[:, :].rearrange("p (j h) -> p j h", h=8)
                nc.vector.tensor_tensor(
                    slgv, sA_s1,
                    elw_t[:, (li - 2) * 64:(li - 1) * 64]
                    .rearrange("p (j h) -> p j h", h=8), ALU.add)
                nc.vector.scalar_tensor_tensor(
                    lgs[:, 64:128].rearrange("p (j h) -> p j h", h=8),
                    slgv, 0.2, slgv, ALU.mult, ALU.max)
            yield
            # f4: exp (both edge and self logits in one ACT op)
            pp = wk.tile([128, 128], vdt, tag="pp")
            if self_loops:
                nc.scalar.activation(pp[:], lgs[:], ACTF.Exp)
            else:
                nc.scalar.activation(pp[0:112, 0:64], lgs[0:112, 0:64],
                                     ACTF.Exp)
            p_e = pp[0:112, 0:64]
            p_self = pp[:, 64:128] if self_loops else None
            yield
            # f5: denominators. L1: per-edge directly via DD (dst
            # coincidence). L2/3: scatter p_e to dst nodes.
            for j in range(8):
                if self_loops:
                    nc.tensor.matmul(psc[:, DEN_ + j * 8:DEN_ + j * 8 + 8],
                                     dtt[j], p_e[0:112, j * 8:j * 8 + 8],
                                     start=True, stop=True)
                else:
                    nc.tensor.matmul(psc[0:112, DEN_ + j * 8:DEN_ + j * 8 + 8],
                                     ddm[j], p_e[0:112, j * 8:j * 8 + 8],
                                     start=True, stop=True)
            yield
            # f6: reciprocal (self prob folded in on DVE)
            if self_loops:
                rv = wk.tile([128, 64], vdt, tag="rv")
                dnm = wk.tile([128, 64], F32, tag="dnm")
                nc.vector.tensor_tensor(dnm[:], psc[:, DEN_:DEN_ + 64],
                                        p_self, ALU.add)
                nc.vector.reciprocal(rv[:], dnm[:])
            else:
                rv = wk.tile([112, 64], vdt, tag="rve")
                nc.vector.reciprocal(rv[:], psc[0:112, DEN_:DEN_ + 64])
            yield
            # f7: gather 1/denom back to edges (L2/3; L1 already per-edge)
            if self_loops:
                for j in range(8):
                    nc.tensor.matmul(
                        psc[0:112, ASD_ + j * 8:ASD_ + j * 8 + 8],
                        dblk[j], rv[:, j * 8:j * 8 + 8],
                        start=True, stop=True)
            yield
            # f8: normalized attention probs
            pn = wk.tile([112, 64], vdt, tag="pn")
            if self_loops:
                nc.vector.tensor_tensor(pn[:], psc[0:112, ASD_:ASD_ + 64],
                                        p_e, ALU.mult)
            else:
                nc.vector.tensor_tensor(pn[:], rv[:], p_e, ALU.mult)
            psn = None
            if self_loops:
                psn = wk.tile([128, 64], vdt, tag="psn")
                nc.gpsimd.tensor_tensor(psn[:], p_self, rv[:], ALU.mult)
            yield
            return {"pn": pn, "psn": psn}

        def octet_program(oct_i):
            # ---------------- A: DMAs (all contiguous octet-major) -------
            sdx = eb.tile([128, 1792], FP8, tag="sdx")
            nc.sync.dma_start(sdx[:], sdfr[oct_i * 128:(oct_i + 1) * 128, :])
            xn_t = eb.tile([128, 128], vdt, tag="xn")
            nc.sync.dma_start(xn_t[:], xnr[oct_i * 128:(oct_i + 1) * 128, :])
            dta = eb.tile([112, 1024], FP8, tag="dta")
            nc.scalar.dma_start(dta[:],
                                dttae[oct_i * 112:(oct_i + 1) * 112, :])
            eaw_t = eb.tile([112, 192], vdt, tag="eaw")
            nc.sync.dma_start(eaw_t[:], eawr[oct_i * 112:(oct_i + 1) * 112, :])
            elw_t = eb.tile([128, 128], vdt, tag="elw")
            nc.sync.dma_start(elw_t[:], elwr[oct_i * 128:(oct_i + 1) * 128, :])
            dd_t = eb.tile([112, 896], FP8, tag="ddm")
            nc.scalar.dma_start(dd_t[:], ddr[oct_i * 112:(oct_i + 1) * 112, :])
            ddm = [dd_t[:, j * 112:(j + 1) * 112] for j in range(8)]
            sblk = [sdx[:, j * 224:j * 224 + 112] for j in range(8)]
            dblk = [sdx[:, j * 224 + 112:j * 224 + 224] for j in range(8)]
            dtt = [dta[0:112, j * 128:(j + 1) * 128] for j in range(8)]
            xn = [xn_t[:, j * 16:(j + 1) * 16] for j in range(8)]
            yield
            yield   # slack for DMA latency

            # =============== layer 1 (gather-first) ===============
            # f0: gather x^T at src+dst endpoints in one matmul per pair
            # (sblk|dblk adjacent in sdx); 2 pairs per psum bank
            xsd_ps = []
            for t in range(4):
                tg = ps.tile([16, 448], F32, tag="big", bufs=4)
                for q2 in range(2):
                    j = t * 2 + q2
                    nc.tensor.matmul(tg[0:16, q2 * 224:q2 * 224 + 224], xn[j],
                                     sdx[:, j * 224:j * 224 + 224],
                                     start=True, stop=True)
                xsd_ps.append(tg)
            yield
            # f1: drain gathered x to bf16
            xsd = []
            for t in range(4):
                tg = wk.tile([16, 448], vdt, tag=f"xsd{t}")
                if t == 1:
                    nc.vector.tensor_copy(tg[:], xsd_ps[t][:])
                else:
                    nc.scalar.copy(tg[:], xsd_ps[t][:])
                xsd.append(tg)

            def xgb(j):
                return xsd[j // 2][0:16, (j % 2) * 224:(j % 2) * 224 + 112]

            def xdb(j):
                return xsd[j // 2][0:16,
                                   (j % 2) * 224 + 112:(j % 2) * 224 + 224]
            yield
            # f2: edge attention scores from gathered x
            psc = ps.tile([128, 336], F32, tag="psc", bufs=2)
            for j in range(8):
                nc.tensor.matmul(psc[0:112, ASD_ + j * 8:ASD_ + j * 8 + 8],
                                 xgb(j), c_w1sr[0:16, 0:8],
                                 start=True, stop=False)
                nc.tensor.matmul(psc[0:112, ASD_ + j * 8:ASD_ + j * 8 + 8],
                                 xdb(j), c_w1sr[0:16, 8:16],
                                 start=False, stop=True)
            yield
            sm = yield from softmax_stages(1, psc, dblk, dtt, None,
                                           eaw_t, elw_t, ddm)
            pn = sm["pn"]
            # f9: project gathered x into messages, apply attention
            msgs = []
            for d in range(4):
                g2 = ps.tile([112, 512], F32, tag="g2", bufs=2)
                for jj in range(2):
                    j = 2 * d + jj
                    nc.tensor.matmul(g2[:, jj * 256:jj * 256 + 256],
                                     xgb(j), c_w1vr[0:16, :],
                                     start=True, stop=True)
                msg2 = wk.tile([112, 512], vdt, tag=f"msg{d}")
                nc.vector.tensor_tensor(
                    msg2[:].rearrange("p (a h c) -> p a h c", a=2, h=H),
                    g2[:, :].rearrange("p (a h c) -> p a h c", a=2, h=H),
                    pn[:, d * 16:(d + 1) * 16]
                    .rearrange("p (a h) -> p a h", a=2)[:, :, :, None]
                    .broadcast_to([112, 2, H, HID]), ALU.mult)
                msgs.append(msg2)
                if d == 1:
                    yield
            yield
            # f10: scatter messages to dst nodes
            o2s = []
            for d in range(4):
                o2 = ps.tile([128, 512], F32, tag="big", bufs=4)
                for jj in range(2):
                    j = 2 * d + jj
                    for c in range(2):
                        cs = jj * 256 + c * 128
                        nc.tensor.matmul(o2[:, cs:cs + 128],
                                         msgs[d][:, cs:cs + 128], dtt[j],
                                         start=True, stop=True)
                o2s.append(o2)
                if d == 1:
                    yield
            yield
            # f11: relu
            prevT = [None] * 8
            for d in range(4):
                hT = wk.tile([128, 512], vdt, tag=f"hT1_{d}", bufs=2)
                nc.scalar.activation(hT[:], o2s[d][:], ACTF.Relu)
                for jj in range(2):
                    j = 2 * d + jj
                    prevT[j] = [hT[:, jj * 256:jj * 256 + 128],
                                hT[:, jj * 256 + 128:jj * 256 + 256]]
                if d == 1:
                    yield
            yield

            # =============== layers 2, 3 ===============
            hT3 = None
            for li in (2, 3):
                # g0: feature projection (values + scores)
                psc = ps.tile([128, 336], F32, tag="psc", bufs=2)
                h2v_l = []
                for d in range(4):
                    h2v = ps.tile([128, 512], F32, tag="big", bufs=4)
                    for jj in range(2):
                        j = 2 * d + jj
                        for kc in range(2):
                            nc.tensor.matmul(h2v[:, jj * 256:jj * 256 + 256],
                                             prevT[j][kc], wv[li][kc],
                                             start=(kc == 0), stop=(kc == 1))
                        for kc in range(2):
                            nc.tensor.matmul(
                                psc[:, SC_ + j * 24:SC_ + j * 24 + 24],
                                prevT[j][kc], wsc[li][kc],
                                start=(kc == 0), stop=(kc == 1))
                    h2v_l.append(h2v)
                    if d == 1:
                        yield
                yield
                # g1: copies to bf16
                hA = []
                for d in range(4):
                    hA_d = wk.tile([128, 512], vdt, tag=f"hA{d}")
                    if d == 1:
                        nc.vector.tensor_copy(hA_d[:], h2v_l[d][:])
                    else:
                        nc.scalar.copy(hA_d[:], h2v_l[d][:])
                    hA.append(hA_d)
                    if d == 1:
                        sA = wk.tile([128, 192], vdt, tag="sA")
                        nc.scalar.copy(sA[:], psc[:, SC_:SC_ + 192])
                        yield
                yield
                # g2s: edge score gathers (ea*we term folded in at f3)
                for j in range(8):
                    nc.tensor.matmul(psc[0:112, ASD_ + j * 8:ASD_ + j * 8 + 8],
                                     sblk[j], sA[:, j * 24:j * 24 + 8],
                                     start=True, stop=False)
                    nc.tensor.matmul(psc[0:112, ASD_ + j * 8:ASD_ + j * 8 + 8],
                                     dblk[j], sA[:, j * 24 + 8:j * 24 + 16],
                                     start=False, stop=True)
                yield
                s1v = sA[:, :].rearrange("p (j c) -> p j c", c=24)[:, :, 16:24]
                sm = yield from softmax_stages(li, psc, dblk, dtt,
                                               s1v, eaw_t, elw_t, ddm)
                pn, psn = sm["pn"], sm["psn"]
                # g9: gather h + apply attention (edge + self messages)
                msgs, msgss = [], []
                for d in range(4):
                    g2 = ps.tile([112, 512], F32, tag="g2", bufs=2)
                    for jj in range(2):
                        j = 2 * d + jj
                        nc.tensor.matmul(g2[:, jj * 256:jj * 256 + 256],
                                         sblk[j],
                                         hA[d][:, jj * 256:jj * 256 + 256],
                                         start=True, stop=True)
                    msg2 = wk.tile([112, 512], vdt, tag=f"msg{d}")
                    if li == 3 and d == 2:
                        g2s = wk.tile([112, 512], vdt, tag="g2s", bufs=2)
                        nc.scalar.copy(g2s[:], g2[:])
                        nc.gpsimd.tensor_tensor(
                            msg2[:].rearrange("p (a h c) -> p a h c", a=2, h=H),
                            g2s[:, :].rearrange("p (a h c) -> p a h c", a=2, h=H),
                            pn[:, d * 16:(d + 1) * 16]
                            .rearrange("p (a h) -> p a h", a=2)[:, :, :, None]
                            .broadcast_to([112, 2, H, HID]), ALU.mult)
                    else:
                        nc.vector.tensor_tensor(
                            msg2[:].rearrange("p (a h c) -> p a h c", a=2, h=H),
                            g2[:, :].rearrange("p (a h c) -> p a h c", a=2, h=H),
                            pn[:, d * 16:(d + 1) * 16]
                            .rearrange("p (a h) -> p a h", a=2)[:, :, :, None]
                            .broadcast_to([112, 2, H, HID]), ALU.mult)
                    msgs.append(msg2)
                    msgs2 = wk.tile([128, 512], vdt, tag=f"msgs{d}")
                    nc.gpsimd.tensor_tensor(
                        msgs2[:].rearrange("p (a h c) -> p a h c", a=2, h=H),
                        hA[d][:, :].rearrange("p (a h c) -> p a h c", a=2, h=H),
                        psn[:, d * 16:(d + 1) * 16]
                        .rearrange("p (a h) -> p a h", a=2)[:, :, :, None]
                        .broadcast_to([128, 2, H, HID]), ALU.mult)
                    msgss.append(msgs2)
                    if d == 1:
                        yield
                yield
                # g10: scatter (edges via dtt, self via identity transpose:
                # msgss is node-major, o2 feature-major, so the id matmul
                # both transposes and accumulates)
                o2s = []
                for d in range(4):
                    o2 = ps.tile([128, 512], F32, tag="big", bufs=4)
                    for jj in range(2):
                        j = 2 * d + jj
                        for c in range(2):
                            cs = jj * 256 + c * 128
                            nc.tensor.matmul(o2[:, cs:cs + 128],
                                             msgs[d][:, cs:cs + 128], dtt[j],
                                             start=True, stop=False)
                            nc.tensor.matmul(o2[:, cs:cs + 128],
                                             msgss[d][:, cs:cs + 128],
                                             c_id[:, :],
                                             start=False, stop=True)
                    o2s.append(o2)
                    if d == 1:
                        yield
                yield
                # g11: relu (into one wide tile so the MLP can batch its
                # agent matmul movers across all 4 d-blocks)
                hTw = wk.tile([128, 2048], vdt, tag=f"hT{li}", bufs=3)
                hT_new = []
                for d in range(4):
                    hT = hTw[:, d * 512:(d + 1) * 512]
                    nc.scalar.activation(hT, o2s[d][:], ACTF.Relu)
                    hT_new.append(hT)
                    for jj in range(2):
                        j = 2 * d + jj
                        prevT[j] = [hTw[:, d * 512 + jj * 256:
                                        d * 512 + jj * 256 + 128],
                                    hTw[:, d * 512 + jj * 256 + 128:
                                        d * 512 + jj * 256 + 256]]
                    if d == 1:
                        yield
                hT3 = hT_new
                hT3w = hTw
                yield

            # =============== MLP head ===============
            # m0: global mean pool partial sums
            gev_o = wk.tile([128, 32], vdt, tag="gev")
            gvv = gev_o[:, :].rearrange("p (c pr g) -> p pr c g", c=2, g=2)
            for d in range(4):
                nc.vector.tensor_reduce(
                    gvv[:, 2 * d:2 * d + 2, :, :],
                    hT3[d][:, :].rearrange(
                        "p (a b g n) -> p a b g n", a=2, b=2, g=2),
                    mybir.AxisListType.X, ALU.add)
                if d == 1:
                    yield
            yield
            # m1: pool to bf16 + agent-feature matmuls (one mover AP spans
            # all 4 d-blocks of the wide hT3 tile)
            zmlp = ps.tile([128, 144], F32, tag="g2", bufs=2)
            gev_v = gev_o
            for c in range(2):
                agent = hT3w[:, :].rearrange(
                    "p (d a b g n) -> p d a b g n",
                    d=4, a=2, b=2, g=2)[:, :, :, c, :, 0:8]
                nc.tensor.matmul(zmlp[:, 0:128],
                                 c_fc1a[:, bass.ts(c, 128)], agent,
                                 start=(c == 0), stop=(c == 1))
            yield
            # m2: graph-embedding matmuls
            for c in range(2):
                nc.tensor.matmul(zmlp[:, 128:144],
                                 c_fc1g[:, bass.ts(c, 128)],
                                 gev_v[:, c * 16:(c + 1) * 16],
                                 start=(c == 0), stop=(c == 1))
            yield
            # m3: biases
            zgb = wk.tile([128, 16], vdt, tag="zgb")
            nc.vector.scalar_tensor_tensor(
                zgb[:], zmlp[:, 128:144], 1.0,
                c_fc1b[:, 0:1].broadcast_to([128, 16]), ALU.mult, ALU.add)
            yield
            zt = wk.tile([128, 128], F32, tag="zt")
            nc.vector.scalar_tensor_tensor(
                zt[:].rearrange("p (a b) -> p a b", a=16),
                zmlp[:, 0:128].rearrange("p (a b) -> p a b", a=16), 1.0,
                zgb[:][:, :, None].broadcast_to([128, 16, 8]),
                ALU.mult, ALU.add)
            yield
            # m4: relu
            zbat = wk.tile([128, 128], vdt, tag="zbat")
            nc.scalar.activation(zbat[:], zt[:], ACTF.Relu)
            yield
            # m5: output projection
            nc.tensor.matmul(zmlp[0:OUT, 0:128], c_fc2w[:, :], zbat[:],
                             start=True, stop=True)
            yield
            # m6: accumulate into output
            nc.vector.tensor_scalar(out_acc[:, oct_i * 128:(oct_i + 1) * 128],
                                    zmlp[0:OUT, 0:128], c_fc2b[:, 0:1], None,
                                    ALU.add)

        # -------- software-pipelined driver: stage-interleave octets --------
        gens = []
        next_o, tick = 0, 0
        while next_o < noct or gens:
            if next_o < noct and tick % start_ivl == 0:
                gens.append(octet_program(next_o))
                next_o += 1
            for g in list(reversed(gens)):
                try:
                    next(g)
                except StopIteration:
                    gens.remove(g)
            if tick == 0:
                # big constants stream in behind octet 0's data
                for t, ap in deferred:
                    nc.sync.dma_start(t[:], ap[:, :])
            tick += 1

        nc.sync.dma_start(out_d[:, :], out_acc[:])

    nc.compile()
    return nc


# ---------------- host-side packing ----------------

def _np_vdt(vdt):
    import ml_dtypes
    return {mybir.dt.bfloat16: ml_dtypes.bfloat16,
            mybir.dt.float32: np.float32}[vdt]


def _np_vdt8():
    return mybir.dt.np(FP8)


def host_prep(inputs, npairs=GPC // 2, vdt=mybir.dt.bfloat16):
    nv = _np_vdt(vdt)
    noct = npairs // 8
    x = np.asarray(inputs["x"], np.float32)
    ei = np.asarray(inputs["edge_index"])
    eattr = np.asarray(inputs["edge_attr"], np.float32).reshape(-1)
    for l in (1, 2, 3):
        assert not np.any(np.asarray(inputs[f"b{l}"])), "GAT bias must be 0"

    def packs(l):
        W = np.asarray(inputs[f"W{l}"], np.float32)
        a_s = np.asarray(inputs[f"as{l}"], np.float32)
        a_d = np.asarray(inputs[f"ad{l}"], np.float32)
        Ps = np.einsum("fkc,kc->fk", W.reshape(W.shape[0], H, HID), a_s)
        Pd = np.einsum("fkc,kc->fk", W.reshape(W.shape[0], H, HID), a_d)
        S = np.concatenate([Ps, Pd, Ps + Pd], axis=1)   # [din, 24]
        return W, S

    def w_e(l):
        We = np.asarray(inputs[f"We{l}"], np.float32).reshape(H, HID)
        a_e = np.asarray(inputs[f"ae{l}"], np.float32)
        return (We * a_e).sum(-1)                       # [H]

    wes_all = np.stack([w_e(1), w_e(2), w_e(3)])        # [3, H]

    W1, S1 = packs(1)
    W2, S2 = packs(2)
    W3, S3 = packs(3)
    w1vr = W1.astype(nv)
    w1sr = S1[:, 0:16].astype(nv)
    w2v = np.concatenate([W2[0:128], W2[128:256]], axis=1).astype(nv)
    w2s = np.concatenate([S2[0:128], S2[128:256]], axis=1).astype(nv)
    w3v = np.concatenate([W3[0:128], W3[128:256]], axis=1).astype(nv)
    w3s = np.concatenate([S3[0:128], S3[128:256]], axis=1).astype(nv)

    fc1_w = np.asarray(inputs["fc1_w"], np.float32)
    fc1a = np.concatenate([fc1_w[:128], fc1_w[128:HC]], axis=1).astype(nv)
    fc1g = np.concatenate([fc1_w[HC:HC + 128] / P,
                           fc1_w[HC + 128:] / P], axis=1).astype(nv)
    fc1b = np.asarray(inputs["fc1_b"], np.float32).reshape(128, 1)
    fc2w = np.asarray(inputs["fc2_w"], np.float32).astype(nv)
    fc2b = np.asarray(inputs["fc2_b"], np.float32).reshape(OUT, 1)
    identm = np.eye(128, dtype=np.float32).astype(nv)

    maps = []
    npc = GPC * P
    epc = GPC * OBS
    for m in range(NCORES):
        nsl = slice(m * npc, (m + 1) * npc)
        esl = slice(m * epc, (m + 1) * epc)
        src = np.asarray(ei[0][esl], np.int64) - m * npc
        dst = np.asarray(ei[1][esl], np.int64) - m * npc
        ea = eattr[esl]
        pairs = np.arange(npairs).repeat(EPP)
        src_l = src - pairs * 128                    # [npairs*112] in [0,128)
        dst_l = dst - pairs * 128
        octs = pairs // 8
        jj = pairs % 8
        epos = np.tile(np.arange(EPP), npairs)

        # sdf: per pair [sblk 112 | dblk 112] one-hots; xn separate (bf16)
        sdf = np.zeros((128, noct, 1792), np.float32)
        sdf[src_l, octs, jj * 224 + epos] = 1.0
        sdf[dst_l, octs, jj * 224 + 112 + epos] = 1.0
        dtt = np.zeros((112, noct, 1024), np.float32)
        dtt[epos, octs, jj * 128 + dst_l] = 1.0

        # dst-coincidence per pair: DD[e', e] = 1 iff dst(e') == dst(e)
        dst_p = dst_l.reshape(npairs, EPP)
        ddm = (dst_p[:, :, None] == dst_p[:, None, :]).astype(np.float32)
        ddm = ddm.reshape(noct, 8, EPP, EPP).transpose(2, 0, 1, 3)

        cnt = np.bincount(dst, minlength=npairs * 128).astype(np.float32)
        easum = np.bincount(dst, weights=ea, minlength=npairs * 128)
        ea_loop = (easum / np.maximum(cnt, 1.0)).astype(np.float32)
        xl = x[nsl].reshape(noct, 8, 128, IN)        # [oct, pair, node, feat]
        xnm = xl.transpose(2, 0, 1, 3).reshape(128, noct, 128)

        # ea*we outer products: eaw[e, (oct, l, j, h)], elw[n, (oct, l, j, h)]
        eacol = ea.reshape(noct, 8, 112).transpose(2, 0, 1)   # [112, oct, j]
        eaw = (eacol[:, :, None, :, None] *
               wes_all[None, None, :, None, :])               # [112,o,3,j,h]
        elcol = ea_loop.reshape(noct, 8, 128).transpose(2, 0, 1)
        elw = (elcol[:, :, None, :, None] *
               wes_all[None, None, 1:, None, :])              # [128,o,2,j,h]

        f8v = np.dtype(_np_vdt8())

        def om(a, p, dt8=False):
            # [p, noct, C] -> octet-major [noct*p, C] for contiguous DMA
            return np.ascontiguousarray(
                a.reshape(p, noct, -1).transpose(1, 0, 2)
            ).reshape(noct * p, -1).astype(f8v if dt8 else nv)

        maps.append({
            "sdfr": om(sdf, 128, True),
            "xnr": om(xnm, 128),
            "dttae": om(dtt, 112, True),
            "ddr": om(ddm.reshape(112, noct * 896), 112, True),
            "eawr": om(eaw.reshape(112, noct * 192), 112),
            "elwr": om(elw.reshape(128, noct * 128), 128),
            "w1vr": w1vr, "w1sr": w1sr,
            "w2v": w2v, "w2s": w2s, "w3v": w3v, "w3s": w3s,
            "fc1a": fc1a, "fc1g": fc1g, "fc1b": fc1b,
            "fc2w": fc2w, "fc2b": fc2b, "ident": identm,
        })
    return maps


def unpack_out(res_list, npairs=GPC // 2):
    outs = []
    for m in range(NCORES):
        o = res_list[m]["out"]
        o = o.reshape(OUT, npairs, 2, A).transpose(1, 2, 3, 0)
        outs.append(o.reshape(npairs * 2, A, OUT))
    return np.concatenate(outs, axis=0).astype(np.float32)


# ---------------- entry point ----------------

LAST_EXEC_NS = None
LAST_RES = None
_NC_CACHE = {}


def kernel(**inputs) -> np.ndarray:
    """Full-input GAT forward on 8 NeuronCores; returns [4096, 8, 2] f32."""
    global LAST_EXEC_NS
    import os
    vdt = mybir.dt.bfloat16
    npairs = GPC // 2
    ivl = int(os.environ.get("BASS_GAT_IVL", START_IVL))
    key = (npairs, vdt, ivl)
    if key not in _NC_CACHE:
        _NC_CACHE[key] = build(npairs, vdt=vdt, num_devices=NCORES,
                               start_ivl=ivl)
    nc = _NC_CACHE[key]
    maps = host_prep(inputs, npairs=npairs, vdt=vdt)
    trace = os.environ.get("BASS_GAT_TRACE") == "1"
    res = None
    for attempt in range(3):
        try:
            res = run_bass_kernel_spmd(
                nc, maps, core_ids=list(range(NCORES)),
                trace=trace and attempt == 0,
                trace_cores=[0] if trace and attempt == 0 else None)
            break
        except Exception:
            if attempt == 2:
                raise
            import time
            time.sleep(10)
    LAST_EXEC_NS = res.exec_time_ns
    global LAST_RES
    LAST_RES = res
    return unpack_out([r for r in res.results], npairs=npairs)



# revision 79
# speedup vs baseline: 1.0409x; 1.0409x over previous
"""GAT model Bass/Tile kernel for TRN2 (self-contained, fine-software-pipelined).

Per core: 512 graphs as 256 pairs (128 nodes / 112 edges per pair),
processed in octets (8 pairs). Each octet's program is split into ~45
fine-grained stages (every cross-engine hop crosses a stage boundary)
and ~4 octets are in flight, staggered START_IVL stages apart, so all
engines stay fed. Layer 1 runs gather-first (gather 16-dim x at edge
endpoints, then project), which skips the layer-1 PSUM h copies.
Graph-structure tensors (one-hot gather/scatter matrices, edge-attr
projections, self-loop means) are host-precomputed and streamed by DMA.
GPSIMD cannot touch PSUM, so PSUM-reading elementwise work is placed on
DVE/ACT and GPSIMD handles SBUF-only multiplies (self messages, logits).
"""
import numpy as np
from contextlib import ExitStack

import concourse.bass as bass
import concourse.tile as tile
from concourse import bacc, mybir
from concourse.bass_utils import run_bass_kernel_spmd

F32 = mybir.dt.float32
I32 = mybir.dt.int32
FP8 = mybir.dt.float8e4

B, A, OBS = 4096, 8, 56
P = 64
H, HID, HC = 8, 32, 256
IN, OUT = 16, 2
NCORES = 8
GPC = B // NCORES
EPP = 2 * OBS              # 112 edges per pair
ALU = mybir.AluOpType
ACTF = mybir.ActivationFunctionType

SC_, ASD_, DEN_ = 0, 208, 272   # psc column regions (f32)
START_IVL = 10


def build(npairs: int, vdt=mybir.dt.bfloat16, num_devices: int = NCORES,
          start_ivl: int = START_IVL):
    assert npairs % 8 == 0
    noct = npairs // 8
    nc = bacc.Bacc("TRN2", target_bir_lowering=False, debug=False,
                   num_devices=num_devices)

    def din(name, shape, dt):
        return nc.dram_tensor(name, shape, dt, kind="ExternalInput").ap()

    # per-octet streams are packed octet-major so each DMA is contiguous;
    # one-hot structure matrices are exact in fp8 (halves their DMA bytes)
    sdfr = din("sdfr", [noct * 128, 1792], FP8)  # [sblk|dblk] x 8 pairs
    xnr = din("xnr", [noct * 128, 128], vdt)     # x per node
    dttae = din("dttae", [noct * 112, 1024], FP8)
    ddr = din("ddr", [noct * 112, 896], FP8)     # dst-coincidence (L1 denom)
    eawr = din("eawr", [noct * 112, 192], vdt)   # ea*we outer products
    elwr = din("elwr", [noct * 128, 128], vdt)   # ea_loop*we outer products
    w1vr = din("w1vr", [IN, 256], vdt)     # W1
    w1sr = din("w1sr", [IN, 16], vdt)      # [Ps1|Pd1]
    w2v = din("w2v", [128, 512], vdt)
    w2s = din("w2s", [128, 48], vdt)
    w3v = din("w3v", [128, 512], vdt)
    w3s = din("w3s", [128, 48], vdt)
    fc1a = din("fc1a", [128, HC], vdt)
    fc1g = din("fc1g", [128, HC], vdt)
    fc1b = din("fc1b", [128, 1], F32)
    fc2w = din("fc2w", [128, OUT], vdt)
    fc2b = din("fc2b", [OUT, 1], F32)
    ident = din("ident", [128, 128], vdt)

    out_d = nc.dram_tensor("out", [OUT, npairs * 16], F32,
                           kind="ExternalOutput").ap()

    with tile.TileContext(nc) as tc, ExitStack() as ctx:
        ctx.enter_context(nc.allow_low_precision(reason="bf16 attention probs"))
        cpool = ctx.enter_context(tc.tile_pool(name="const", bufs=1))
        wk = ctx.enter_context(tc.tile_pool(name="work", bufs=4))
        eb = ctx.enter_context(tc.tile_pool(name="edges", bufs=6))
        ps = ctx.enter_context(tc.tile_pool(name="psum", bufs=1, space="PSUM"))

        def cload(ap, tag):
            t = cpool.tile(list(ap.shape), ap.dtype, tag=tag)
            nc.sync.dma_start(t[:], ap[:, :])
            return t

        deferred = []

        def cload_deferred(ap, tag):
            # alloc now, DMA after octet 0's stream DMAs so the big
            # constants (not needed until mid-octet) don't delay the
            # pipeline fill on the DMA queues
            t = cpool.tile(list(ap.shape), ap.dtype, tag=tag)
            deferred.append((t, ap))
            return t

        c_w1vr, c_w1sr = cload(w1vr, "w1vr"), cload(w1sr, "w1sr")
        c_w2v, c_w2s = cload_deferred(w2v, "w2v"), cload_deferred(w2s, "w2s")
        c_w3v, c_w3s = cload_deferred(w3v, "w3v"), cload_deferred(w3s, "w3s")
        c_fc1a, c_fc1g = (cload_deferred(fc1a, "fc1a"),
                          cload_deferred(fc1g, "fc1g"))
        c_fc1b, c_fc2w, c_fc2b = (cload(fc1b, "fc1b"), cload(fc2w, "fc2w"),
                                  cload(fc2b, "fc2b"))
        c_id = cload_deferred(ident, "ident")

        out_acc = cpool.tile([OUT, npairs * 16], F32, tag="out_acc")

        wv = {2: [c_w2v[:, 0:256], c_w2v[:, 256:512]],
              3: [c_w3v[:, 0:256], c_w3v[:, 256:512]]}
        wsc = {2: [c_w2s[:, 0:24], c_w2s[:, 24:48]],
               3: [c_w3s[:, 0:24], c_w3s[:, 24:48]]}

        def softmax_stages(li, psc, dblk, dtt, sA_s1, eaw_t, elw_t, ddm):
            """Stages f3..f8 shared by all layers: logits -> pn/psn.
            The ea*we edge-attr term is host-precomputed (eaw/elw) and
            added on DVE here instead of rank-1 PE matmuls. For L1 (no
            self loops) per-edge denominators come from one DD matmul
            (dst-coincidence), skipping the scatter+gather pair."""
            self_loops = li > 1
            # f3: add ea*we term + leaky-relu logits; exp'd in one op
            lgs = wk.tile([128, 128], vdt, tag="lgs")
            lg = wk.tile([112, 64], vdt, tag="lg")
            nc.vector.tensor_tensor(
                lg[:], psc[0:112, ASD_:ASD_ + 64],
                eaw_t[0:112, (li - 1) * 64:li * 64], ALU.add)
            nc.vector.scalar_tensor_tensor(
                lgs[0:112, 0:64], lg[:], 0.2, lg[:], ALU.mult, ALU.max)
            if self_loops:
                slg = wk.tile([128, 64], vdt, tag="slg")
                slgv = slg[:, :].rearrange("p (j h) -> p j h", h=8)
                nc.vector.tensor_tensor(
                    slgv, sA_s1,
                    elw_t[:, (li - 2) * 64:(li - 1) * 64]
                    .rearrange("p (j h) -> p j h", h=8), ALU.add)
                nc.vector.scalar_tensor_tensor(
                    lgs[:, 64:128].rearrange("p (j h) -> p j h", h=8),
                    slgv, 0.2, slgv, ALU.mult, ALU.max)
            yield
            # f4: exp (both edge and self logits in one ACT op)
            pp = wk.tile([128, 128], vdt, tag="pp")
            if self_loops:
                nc.scalar.activation(pp[:], lgs[:], ACTF.Exp)
            else:
                nc.scalar.activation(pp[0:112, 0:64], lgs[0:112, 0:64],
                                     ACTF.Exp)
            p_e = pp[0:112, 0:64]
            p_self = pp[:, 64:128] if self_loops else None
            yield
            # f5: denominators. L1: per-edge directly via DD (dst
            # coincidence). L2/3: scatter p_e to dst nodes.
            for j in range(8):
                if self_loops:
                    nc.tensor.matmul(psc[:, DEN_ + j * 8:DEN_ + j * 8 + 8],
                                     dtt[j], p_e[0:112, j * 8:j * 8 + 8],
                                     start=True, stop=True)
                else:
                    nc.tensor.matmul(psc[0:112, DEN_ + j * 8:DEN_ + j * 8 + 8],
                                     ddm[j], p_e[0:112, j * 8:j * 8 + 8],
                                     start=True, stop=True)
            yield
            # f6: reciprocal (self prob folded in on DVE)
            if self_loops:
                rv = wk.tile([128, 64], vdt, tag="rv")
                dnm = wk.tile([128, 64], F32, tag="dnm")
                nc.vector.tensor_tensor(dnm[:], psc[:, DEN_:DEN_ + 64],
                                        p_self, ALU.add)
                nc.vector.reciprocal(rv[:], dnm[:])
            else:
                rv = wk.tile([112, 64], vdt, tag="rve")
                nc.vector.reciprocal(rv[:], psc[0:112, DEN_:DEN_ + 64])
            yield
            # f7: gather 1/denom back to edges (L2/3; L1 already per-edge)
            if self_loops:
                for j in range(8):
                    nc.tensor.matmul(
                        psc[0:112, ASD_ + j * 8:ASD_ + j * 8 + 8],
                        dblk[j], rv[:, j * 8:j * 8 + 8],
                        start=True, stop=True)
            yield
            # f8: normalized attention probs
            pn = wk.tile([112, 64], vdt, tag="pn")
            if self_loops:
                nc.vector.tensor_tensor(pn[:], psc[0:112, ASD_:ASD_ + 64],
                                        p_e, ALU.mult)
            else:
                nc.vector.tensor_tensor(pn[:], rv[:], p_e, ALU.mult)
            psn = None
            if self_loops:
                psn = wk.tile([128, 64], vdt, tag="psn")
                nc.gpsimd.tensor_tensor(psn[:], p_self, rv[:], ALU.mult)
            yield
            return {"pn": pn, "psn": psn}

        def octet_program(oct_i):
            # ---------------- A: DMAs (all contiguous octet-major) -------
            sdx = eb.tile([128, 1792], FP8, tag="sdx")
            nc.sync.dma_start(sdx[:], sdfr[oct_i * 128:(oct_i + 1) * 128, :])
            xn_t = eb.tile([128, 128], vdt, tag="xn")
            nc.sync.dma_start(xn_t[:], xnr[oct_i * 128:(oct_i + 1) * 128, :])
            dta = eb.tile([112, 1024], FP8, tag="dta")
            nc.sync.dma_start(dta[:], dttae[oct_i * 112:(oct_i + 1) * 112, :])
            eaw_t = eb.tile([112, 192], vdt, tag="eaw")
            nc.sync.dma_start(eaw_t[:], eawr[oct_i * 112:(oct_i + 1) * 112, :])
            elw_t = eb.tile([128, 128], vdt, tag="elw")
            nc.sync.dma_start(elw_t[:], elwr[oct_i * 128:(oct_i + 1) * 128, :])
            dd_t = eb.tile([112, 896], FP8, tag="ddm")
            nc.sync.dma_start(dd_t[:], ddr[oct_i * 112:(oct_i + 1) * 112, :])
            ddm = [dd_t[:, j * 112:(j + 1) * 112] for j in range(8)]
            sblk = [sdx[:, j * 224:j * 224 + 112] for j in range(8)]
            dblk = [sdx[:, j * 224 + 112:j * 224 + 224] for j in range(8)]
            dtt = [dta[0:112, j * 128:(j + 1) * 128] for j in range(8)]
            xn = [xn_t[:, j * 16:(j + 1) * 16] for j in range(8)]
            yield
            yield   # slack for DMA latency

            # =============== layer 1 (gather-first) ===============
            # f0: gather x^T at src+dst endpoints in one matmul per pair
            # (sblk|dblk adjacent in sdx); 2 pairs per psum bank
            xsd_ps = []
            for t in range(4):
                tg = ps.tile([16, 448], F32, tag="big", bufs=4)
                for q2 in range(2):
                    j = t * 2 + q2
                    nc.tensor.matmul(tg[0:16, q2 * 224:q2 * 224 + 224], xn[j],
                                     sdx[:, j * 224:j * 224 + 224],
                                     start=True, stop=True)
                xsd_ps.append(tg)
            yield
            # f1: drain gathered x to bf16
            xsd = []
            for t in range(4):
                tg = wk.tile([16, 448], vdt, tag=f"xsd{t}")
                if t == 1:
                    nc.vector.tensor_copy(tg[:], xsd_ps[t][:])
                else:
                    nc.scalar.copy(tg[:], xsd_ps[t][:])
                xsd.append(tg)

            def xgb(j):
                return xsd[j // 2][0:16, (j % 2) * 224:(j % 2) * 224 + 112]

            def xdb(j):
                return xsd[j // 2][0:16,
                                   (j % 2) * 224 + 112:(j % 2) * 224 + 224]
            yield
            # f2: edge attention scores from gathered x
            psc = ps.tile([128, 336], F32, tag="psc", bufs=2)
            for j in range(8):
                nc.tensor.matmul(psc[0:112, ASD_ + j * 8:ASD_ + j * 8 + 8],
                                 xgb(j), c_w1sr[0:16, 0:8],
                                 start=True, stop=False)
                nc.tensor.matmul(psc[0:112, ASD_ + j * 8:ASD_ + j * 8 + 8],
                                 xdb(j), c_w1sr[0:16, 8:16],
                                 start=False, stop=True)
            yield
            sm = yield from softmax_stages(1, psc, dblk, dtt, None,
                                           eaw_t, elw_t, ddm)
            pn = sm["pn"]
            # f9: project gathered x into messages, apply attention
            msgs = []
            for d in range(4):
                g2 = ps.tile([112, 512], F32, tag="g2", bufs=2)
                for jj in range(2):
                    j = 2 * d + jj
                    nc.tensor.matmul(g2[:, jj * 256:jj * 256 + 256],
                                     xgb(j), c_w1vr[0:16, :],
                                     start=True, stop=True)
                msg2 = wk.tile([112, 512], vdt, tag=f"msg{d}")
                nc.vector.tensor_tensor(
                    msg2[:].rearrange("p (a h c) -> p a h c", a=2, h=H),
                    g2[:, :].rearrange("p (a h c) -> p a h c", a=2, h=H),
                    pn[:, d * 16:(d + 1) * 16]
                    .rearrange("p (a h) -> p a h", a=2)[:, :, :, None]
                    .broadcast_to([112, 2, H, HID]), ALU.mult)
                msgs.append(msg2)
                if d == 1:
                    yield
            yield
            # f10: scatter messages to dst nodes
            o2s = []
            for d in range(4):
                o2 = ps.tile([128, 512], F32, tag="big", bufs=4)
                for jj in range(2):
                    j = 2 * d + jj
                    for c in range(2):
                        cs = jj * 256 + c * 128
                        nc.tensor.matmul(o2[:, cs:cs + 128],
                                         msgs[d][:, cs:cs + 128], dtt[j],
                                         start=True, stop=True)
                o2s.append(o2)
                if d == 1:
                    yield
            yield
            # f11: relu
            prevT = [None] * 8
            for d in range(4):
                hT = wk.tile([128, 512], vdt, tag=f"hT1_{d}", bufs=2)
                nc.scalar.activation(hT[:], o2s[d][:], ACTF.Relu)
                for jj in range(2):
                    j = 2 * d + jj
                    prevT[j] = [hT[:, jj * 256:jj * 256 + 128],
                                hT[:, jj * 256 + 128:jj * 256 + 256]]
                if d == 1:
                    yield
            yield

            # =============== layers 2, 3 ===============
            hT3 = None
            for li in (2, 3):
                # g0: feature projection (values + scores)
                psc = ps.tile([128, 336], F32, tag="psc", bufs=2)
                h2v_l = []
                for d in range(4):
                    h2v = ps.tile([128, 512], F32, tag="big", bufs=4)
                    for jj in range(2):
                        j = 2 * d + jj
                        for kc in range(2):
                            nc.tensor.matmul(h2v[:, jj * 256:jj * 256 + 256],
                                             prevT[j][kc], wv[li][kc],
                                             start=(kc == 0), stop=(kc == 1))
                        for kc in range(2):
                            nc.tensor.matmul(
                                psc[:, SC_ + j * 24:SC_ + j * 24 + 24],
                                prevT[j][kc], wsc[li][kc],
                                start=(kc == 0), stop=(kc == 1))
                    h2v_l.append(h2v)
                    if d == 1:
                        yield
                yield
                # g1: copies to bf16
                hA = []
                for d in range(4):
                    hA_d = wk.tile([128, 512], vdt, tag=f"hA{d}")
                    if d == 1:
                        nc.vector.tensor_copy(hA_d[:], h2v_l[d][:])
                    else:
                        nc.scalar.copy(hA_d[:], h2v_l[d][:])
                    hA.append(hA_d)
                    if d == 1:
                        sA = wk.tile([128, 192], vdt, tag="sA")
                        nc.scalar.copy(sA[:], psc[:, SC_:SC_ + 192])
                        yield
                yield
                # g2s: edge score gathers (ea*we term folded in at f3)
                for j in range(8):
                    nc.tensor.matmul(psc[0:112, ASD_ + j * 8:ASD_ + j * 8 + 8],
                                     sblk[j], sA[:, j * 24:j * 24 + 8],
                                     start=True, stop=False)
                    nc.tensor.matmul(psc[0:112, ASD_ + j * 8:ASD_ + j * 8 + 8],
                                     dblk[j], sA[:, j * 24 + 8:j * 24 + 16],
                                     start=False, stop=True)
                yield
                s1v = sA[:, :].rearrange("p (j c) -> p j c", c=24)[:, :, 16:24]
                sm = yield from softmax_stages(li, psc, dblk, dtt,
                                               s1v, eaw_t, elw_t, ddm)
                pn, psn = sm["pn"], sm["psn"]
                # g9: gather h + apply attention (edge + self messages)
                msgs, msgss = [], []
                for d in range(4):
                    g2 = ps.tile([112, 512], F32, tag="g2", bufs=2)
                    for jj in range(2):
                        j = 2 * d + jj
                        nc.tensor.matmul(g2[:, jj * 256:jj * 256 + 256],
                                         sblk[j],
                                         hA[d][:, jj * 256:jj * 256 + 256],
                                         start=True, stop=True)
                    msg2 = wk.tile([112, 512], vdt, tag=f"msg{d}")
                    if li == 3 and d == 2:
                        g2s = wk.tile([112, 512], vdt, tag="g2s", bufs=2)
                        nc.scalar.copy(g2s[:], g2[:])
                        nc.gpsimd.tensor_tensor(
                            msg2[:].rearrange("p (a h c) -> p a h c", a=2, h=H),
                            g2s[:, :].rearrange("p (a h c) -> p a h c", a=2, h=H),
                            pn[:, d * 16:(d + 1) * 16]
                            .rearrange("p (a h) -> p a h", a=2)[:, :, :, None]
                            .broadcast_to([112, 2, H, HID]), ALU.mult)
                    else:
                        nc.vector.tensor_tensor(
                            msg2[:].rearrange("p (a h c) -> p a h c", a=2, h=H),
                            g2[:, :].rearrange("p (a h c) -> p a h c", a=2, h=H),
                            pn[:, d * 16:(d + 1) * 16]
                            .rearrange("p (a h) -> p a h", a=2)[:, :, :, None]
                            .broadcast_to([112, 2, H, HID]), ALU.mult)
                    msgs.append(msg2)
                    msgs2 = wk.tile([128, 512], vdt, tag=f"msgs{d}")
                    nc.gpsimd.tensor_tensor(
                        msgs2[:].rearrange("p (a h c) -> p a h c", a=2, h=H),
                        hA[d][:, :].rearrange("p (a h c) -> p a h c", a=2, h=H),
                        psn[:, d * 16:(d + 1) * 16]
                        .rearrange("p (a h) -> p a h", a=2)[:, :, :, None]
                        .broadcast_to([128, 2, H, HID]), ALU.mult)
                    msgss.append(msgs2)
                    if d == 1:
                        yield
                yield
                # g10: scatter (edges via dtt, self via identity transpose:
                # msgss is node-major, o2 feature-major, so the id matmul
                # both transposes and accumulates)
                o2s = []
                for d in range(4):
                    o2 = ps.tile([128, 512], F32, tag="big", bufs=4)
                    for jj in range(2):
                        j = 2 * d + jj
                        for c in range(2):
                            cs = jj * 256 + c * 128
                            nc.tensor.matmul(o2[:, cs:cs + 128],
                                             msgs[d][:, cs:cs + 128], dtt[j],
                                             start=True, stop=False)
                            nc.tensor.matmul(o2[:, cs:cs + 128],
                                             msgss[d][:, cs:cs + 128],
                                             c_id[:, :],
                                             start=False, stop=True)
                    o2s.append(o2)
                    if d == 1:
                        yield
                yield
                # g11: relu (into one wide tile so the MLP can batch its
                # agent matmul movers across all 4 d-blocks)
                hTw = wk.tile([128, 2048], vdt, tag=f"hT{li}", bufs=3)
                hT_new = []
                for d in range(4):
                    hT = hTw[:, d * 512:(d + 1) * 512]
                    nc.scalar.activation(hT, o2s[d][:], ACTF.Relu)
                    hT_new.append(hT)
                    for jj in range(2):
                        j = 2 * d + jj
                        prevT[j] = [hTw[:, d * 512 + jj * 256:
                                        d * 512 + jj * 256 + 128],
                                    hTw[:, d * 512 + jj * 256 + 128:
                                        d * 512 + jj * 256 + 256]]
                    if d == 1:
                        yield
                hT3 = hT_new
                hT3w = hTw
                yield

            # =============== MLP head ===============
            # m0: global mean pool partial sums
            gev_o = wk.tile([128, 32], vdt, tag="gev")
            gvv = gev_o[:, :].rearrange("p (c pr g) -> p pr c g", c=2, g=2)
            for d in range(4):
                nc.vector.tensor_reduce(
                    gvv[:, 2 * d:2 * d + 2, :, :],
                    hT3[d][:, :].rearrange(
                        "p (a b g n) -> p a b g n", a=2, b=2, g=2),
                    mybir.AxisListType.X, ALU.add)
                if d == 1:
                    yield
            yield
            # m1: pool to bf16 + agent-feature matmuls (one mover AP spans
            # all 4 d-blocks of the wide hT3 tile)
            zmlp = ps.tile([128, 144], F32, tag="g2", bufs=2)
            gev_v = gev_o
            for c in range(2):
                agent = hT3w[:, :].rearrange(
                    "p (d a b g n) -> p d a b g n",
                    d=4, a=2, b=2, g=2)[:, :, :, c, :, 0:8]
                nc.tensor.matmul(zmlp[:, 0:128],
                                 c_fc1a[:, bass.ts(c, 128)], agent,
                                 start=(c == 0), stop=(c == 1))
            yield
            # m2: graph-embedding matmuls
            for c in range(2):
                nc.tensor.matmul(zmlp[:, 128:144],
                                 c_fc1g[:, bass.ts(c, 128)],
                                 gev_v[:, c * 16:(c + 1) * 16],
                                 start=(c == 0), stop=(c == 1))
            yield
            # m3: biases
            zgb = wk.tile([128, 16], vdt, tag="zgb")
            nc.vector.scalar_tensor_tensor(
                zgb[:], zmlp[:, 128:144], 1.0,
                c_fc1b[:, 0:1].broadcast_to([128, 16]), ALU.mult, ALU.add)
            yield
            zt = wk.tile([128, 128], F32, tag="zt")
            nc.vector.scalar_tensor_tensor(
                zt[:].rearrange("p (a b) -> p a b", a=16),
                zmlp[:, 0:128].rearrange("p (a b) -> p a b", a=16), 1.0,
                zgb[:][:, :, None].broadcast_to([128, 16, 8]),
                ALU.mult, ALU.add)
            yield
            # m4: relu
            zbat = wk.tile([128, 128], vdt, tag="zbat")
            nc.scalar.activation(zbat[:], zt[:], ACTF.Relu)
            yield
            # m5: output projection
            nc.tensor.matmul(zmlp[0:OUT, 0:128], c_fc2w[:, :], zbat[:],
                             start=True, stop=True)
            yield
            # m6: accumulate into output
            nc.vector.tensor_scalar(out_acc[:, oct_i * 128:(oct_i + 1) * 128],
                                    zmlp[0:OUT, 0:128], c_fc2b[:, 0:1], None,
                                    ALU.add)

        # -------- software-pipelined driver: stage-interleave octets --------
        gens = []
        next_o, tick = 0, 0
        while next_o < noct or gens:
            if next_o < noct and tick % start_ivl == 0:
                gens.append(octet_program(next_o))
                next_o += 1
            for g in list(reversed(gens)):
                try:
                    next(g)
                except StopIteration:
                    gens.remove(g)
            if tick == 0:
                # big constants stream in behind octet 0's data
                for t, ap in deferred:
                    nc.sync.dma_start(t[:], ap[:, :])
            tick += 1

        nc.sync.dma_start(out_d[:, :], out_acc[:])

    nc.compile()
    return nc


# ---------------- host-side packing ----------------

def _np_vdt(vdt):
    import ml_dtypes
    return {mybir.dt.bfloat16: ml_dtypes.bfloat16,
            mybir.dt.float32: np.float32}[vdt]


def _np_vdt8():
    return mybir.dt.np(FP8)


def host_prep(inputs, npairs=GPC // 2, vdt=mybir.dt.bfloat16):
    nv = _np_vdt(vdt)
    noct = npairs // 8
    x = np.asarray(inputs["x"], np.float32)
    ei = np.asarray(inputs["edge_index"])
    eattr = np.asarray(inputs["edge_attr"], np.float32).reshape(-1)
    for l in (1, 2, 3):
        assert not np.any(np.asarray(inputs[f"b{l}"])), "GAT bias must be 0"

    def packs(l):
        W = np.asarray(inputs[f"W{l}"], np.float32)
        a_s = np.asarray(inputs[f"as{l}"], np.float32)
        a_d = np.asarray(inputs[f"ad{l}"], np.float32)
        Ps = np.einsum("fkc,kc->fk", W.reshape(W.shape[0], H, HID), a_s)
        Pd = np.einsum("fkc,kc->fk", W.reshape(W.shape[0], H, HID), a_d)
        S = np.concatenate([Ps, Pd, Ps + Pd], axis=1)   # [din, 24]
        return W, S

    def w_e(l):
        We = np.asarray(inputs[f"We{l}"], np.float32).reshape(H, HID)
        a_e = np.asarray(inputs[f"ae{l}"], np.float32)
        return (We * a_e).sum(-1)                       # [H]

    wes_all = np.stack([w_e(1), w_e(2), w_e(3)])        # [3, H]

    W1, S1 = packs(1)
    W2, S2 = packs(2)
    W3, S3 = packs(3)
    w1vr = W1.astype(nv)
    w1sr = S1[:, 0:16].astype(nv)
    w2v = np.concatenate([W2[0:128], W2[128:256]], axis=1).astype(nv)
    w2s = np.concatenate([S2[0:128], S2[128:256]], axis=1).astype(nv)
    w3v = np.concatenate([W3[0:128], W3[128:256]], axis=1).astype(nv)
    w3s = np.concatenate([S3[0:128], S3[128:256]], axis=1).astype(nv)

    fc1_w = np.asarray(inputs["fc1_w"], np.float32)
    fc1a = np.concatenate([fc1_w[:128], fc1_w[128:HC]], axis=1).astype(nv)
    fc1g = np.concatenate([fc1_w[HC:HC + 128] / P,
                           fc1_w[HC + 128:] / P], axis=1).astype(nv)
    fc1b = np.asarray(inputs["fc1_b"], np.float32).reshape(128, 1)
    fc2w = np.asarray(inputs["fc2_w"], np.float32).astype(nv)
    fc2b = np.asarray(inputs["fc2_b"], np.float32).reshape(OUT, 1)
    identm = np.eye(128, dtype=np.float32).astype(nv)

    maps = []
    npc = GPC * P
    epc = GPC * OBS
    for m in range(NCORES):
        nsl = slice(m * npc, (m + 1) * npc)
        esl = slice(m * epc, (m + 1) * epc)
        src = np.asarray(ei[0][esl], np.int64) - m * npc
        dst = np.asarray(ei[1][esl], np.int64) - m * npc
        ea = eattr[esl]
        pairs = np.arange(npairs).repeat(EPP)
        src_l = src - pairs * 128                    # [npairs*112] in [0,128)
        dst_l = dst - pairs * 128
        octs = pairs // 8
        jj = pairs % 8
        epos = np.tile(np.arange(EPP), npairs)

        # sdf: per pair [sblk 112 | dblk 112] one-hots; xn separate (bf16)
        sdf = np.zeros((128, noct, 1792), np.float32)
        sdf[src_l, octs, jj * 224 + epos] = 1.0
        sdf[dst_l, octs, jj * 224 + 112 + epos] = 1.0
        dtt = np.zeros((112, noct, 1024), np.float32)
        dtt[epos, octs, jj * 128 + dst_l] = 1.0

        # dst-coincidence per pair: DD[e', e] = 1 iff dst(e') == dst(e)
        dst_p = dst_l.reshape(npairs, EPP)
        ddm = (dst_p[:, :, None] == dst_p[:, None, :]).astype(np.float32)
        ddm = ddm.reshape(noct, 8, EPP, EPP).transpose(2, 0, 1, 3)

        cnt = np.bincount(dst, minlength=npairs * 128).astype(np.float32)
        easum = np.bincount(dst, weights=ea, minlength=npairs * 128)
        ea_loop = (easum / np.maximum(cnt, 1.0)).astype(np.float32)
        xl = x[nsl].reshape(noct, 8, 128, IN)        # [oct, pair, node, feat]
        xnm = xl.transpose(2, 0, 1, 3).reshape(128, noct, 128)

        # ea*we outer products: eaw[e, (oct, l, j, h)], elw[n, (oct, l, j, h)]
        eacol = ea.reshape(noct, 8, 112).transpose(2, 0, 1)   # [112, oct, j]
        eaw = (eacol[:, :, None, :, None] *
               wes_all[None, None, :, None, :])               # [112,o,3,j,h]
        elcol = ea_loop.reshape(noct, 8, 128).transpose(2, 0, 1)
        elw = (elcol[:, :, None, :, None] *
               wes_all[None, None, 1:, None, :])              # [128,o,2,j,h]

        f8v = np.dtype(_np_vdt8())

        def om(a, p, dt8=False):
            # [p, noct, C] -> octet-major [noct*p, C] for contiguous DMA
            return np.ascontiguousarray(
                a.reshape(p, noct, -1).transpose(1, 0, 2)
            ).reshape(noct * p, -1).astype(f8v if dt8 else nv)

        maps.append({
            "sdfr": om(sdf, 128, True),
            "xnr": om(xnm, 128),
            "dttae": om(dtt, 112, True),
            "ddr": om(ddm.reshape(112, noct * 896), 112, True),
            "eawr": om(eaw.reshape(112, noct * 192), 112),
            "elwr": om(elw.reshape(128, noct * 128), 128),
            "w1vr": w1vr, "w1sr": w1sr,
            "w2v": w2v, "w2s": w2s, "w3v": w3v, "w3s": w3s,
            "fc1a": fc1a, "fc1g": fc1g, "fc1b": fc1b,
            "fc2w": fc2w, "fc2b": fc2b, "ident": identm,
        })
    return maps


def unpack_out(res_list, npairs=GPC // 2):
    outs = []
    for m in range(NCORES):
        o = res_list[m]["out"]
        o = o.reshape(OUT, npairs, 2, A).transpose(1, 2, 3, 0)
        outs.append(o.reshape(npairs * 2, A, OUT))
    return np.concatenate(outs, axis=0).astype(np.float32)


# ---------------- entry point ----------------

LAST_EXEC_NS = None
LAST_RES = None
_NC_CACHE = {}


def kernel(**inputs) -> np.ndarray:
    """Full-input GAT forward on 8 NeuronCores; returns [4096, 8, 2] f32."""
    global LAST_EXEC_NS
    import os
    vdt = mybir.dt.bfloat16
    npairs = GPC // 2
    ivl = int(os.environ.get("BASS_GAT_IVL", START_IVL))
    key = (npairs, vdt, ivl)
    if key not in _NC_CACHE:
        _NC_CACHE[key] = build(npairs, vdt=vdt, num_devices=NCORES,
                               start_ivl=ivl)
    nc = _NC_CACHE[key]
    maps = host_prep(inputs, npairs=npairs, vdt=vdt)
    trace = os.environ.get("BASS_GAT_TRACE") == "1"
    res = None
    for attempt in range(3):
        try:
            res = run_bass_kernel_spmd(
                nc, maps, core_ids=list(range(NCORES)),
                trace=trace and attempt == 0,
                trace_cores=[0] if trace and attempt == 0 else None)
            break
        except Exception:
            if attempt == 2:
                raise
            import time
            time.sleep(10)
    LAST_EXEC_NS = res.exec_time_ns
    global LAST_RES
    LAST_RES = res
    return unpack_out([r for r in res.results], npairs=npairs)

